# revision 12
# baseline (speedup 1.0000x reference)
"""Trainium2 Bass kernel for CNNLayer: conv(K=3 along H) + bias + tanh + topk(50) along H.

Full input contract:
  x:      [1024, 1, 200, 32] f32
  conv_w: [16, 1, 3, 1]      f32
  conv_b: [16]               f32
Output: [1024, 16, 50, 32] f32 (top-50 along H, sorted descending, after tanh)

Strategy: data-parallel over batch across 8 cores (128 samples/core).
Per sample:
  - load x[s] in h32-partition layout, DVE stream-transpose to xT[w, h]
  - replicate-shift xT into xT3b[(k,w8), (wb, c)] via 12 small SBUF DMAs
    (c = h+1; tap shifts baked into column offsets; boundary cols zeroed)
  - conv as 2 PE matmuls contracting K=24=(k,w8) with block-diagonal weights
    -> z[(o,w8), (wb, h)] in PSUM (no padding, no accumulation groups)
  - top-k on raw z (tanh monotonic => topk commutes): 7x DVE max8 + 6x
    match_replace masking per (o,w8)-row over the 200 h values
  - tanh(top50 + bias[o]) on ACT
  - PE-transpose result to [r, (o,w8)] so the output DMA is w-contiguous
"""

import sys

sys.path.insert(0, "/opt/trn_rl_repo")

import numpy as np

N_CORES = 8
B, H, W = 1024, 200, 32
COUT, KH, TOPK = 16, 3, 50
BS = B // N_CORES  # samples per core
NWB = W // 8       # 4 w-blocks of 8 -> (o, w8) = 128 partition rows
NV = 56            # values extracted per row (7 rounds of 8)
CW = 202           # columns per wb block in xT3b (c = h+1, c in [0, 201])

_CACHE = {}


def build_module(n_samples=BS):
    import concourse.bass as bass  # noqa: F401
    import concourse.tile as tile
    from concourse import bacc, mybir

    f32 = mybir.dt.float32
    nc = bacc.Bacc("TRN2", target_bir_lowering=False, debug=False,
                   num_devices=N_CORES)

    x = nc.dram_tensor("x", [n_samples, H, W], f32, kind="ExternalInput").ap()
    wkj = nc.dram_tensor("wkj", [KH * 8, 128], f32, kind="ExternalInput").ap()
    bias = nc.dram_tensor("bias_p", [128, 1], f32, kind="ExternalInput").ap()
    ident = nc.dram_tensor("ident", [128, 128], f32, kind="ExternalInput").ap()
    out = nc.dram_tensor("out", [n_samples, COUT, TOPK, W], f32,
                         kind="ExternalOutput").ap()

    with tile.TileContext(nc) as tc:
        with (
            tc.tile_pool(name="const", bufs=1) as constp,
            tc.tile_pool(name="xin", bufs=3) as xinp,
            tc.tile_pool(name="xt", bufs=3) as xtp,
            tc.tile_pool(name="xt3", bufs=3) as xt3p,
            tc.tile_pool(name="zpsum", bufs=4, space="PSUM") as zpsum,
            tc.tile_pool(name="zs", bufs=3) as zsp,
            tc.tile_pool(name="v", bufs=4) as vp,
            tc.tile_pool(name="res", bufs=4) as resp,
            tc.tile_pool(name="otpsum", bufs=2, space="PSUM") as otpsum,
            tc.tile_pool(name="u", bufs=3) as up,
        ):
            wk_sb = []
            for k in range(KH):
                wt = constp.tile([8, 128], f32, tag=f"wk{k}")
                nc.sync.dma_start(wt[:], wkj[8 * k:8 * (k + 1), :])
                wk_sb.append(wt)
            ident_sb = constp.tile([128, 128], f32)
            nc.sync.dma_start(ident_sb[:], ident[:])
            bias_sb = constp.tile([128, 1], f32)
            nc.sync.dma_start(bias_sb[:], bias[:])

            for s in range(n_samples):
                # load x[s]=[200,32] as [h32, (hb, w)]: 6 blocks + tail(h 168:200)
                xin = xinp.tile([32, 7 * 32], f32)
                nc.sync.dma_start(
                    xin[:, 0:192].rearrange("p (hb w) -> p hb w", w=32),
                    x[s, 0:192].rearrange("(hb p) w -> p hb w", p=32),
                )
                nc.sync.dma_start(xin[:, 192:224], x[s, 168:200])
                # DVE 32x32 block transpose -> xT[w, xrow]
                xT = xtp.tile([32, H], f32)
                nc.vector.transpose(xT[:, 0:192], xin[:, 0:192])
                nc.vector.transpose(xT[:, 168:200], xin[:, 192:224])

                # xT8[w8, (wb, j)] = x[j, 8wb+w8]: w-blocks into the free dim
                x8 = xt3p.tile([8, NWB * H], f32)
                for wb in range(NWB):
                    nc.sync.dma_start(x8[:, H * wb:H * (wb + 1)],
                                      xT[8 * wb:8 * wb + 8, :])

                # conv per wb: 3 accumulating matmuls (taps), K=8;
                # per-tap column ranges make the H-boundary exact (no pad)
                zs = zsp.tile([128, NWB * H], f32)
                for wb in range(NWB):
                    z = zpsum.tile([128, H], f32)
                    xv = x8[:, H * wb:H * (wb + 1)]
                    nc.tensor.matmul(z[:, 0:H], wk_sb[1][:],
                                     xv[:, 0:H], start=True, stop=False)
                    nc.tensor.matmul(z[:, 1:H], wk_sb[0][:],
                                     xv[:, 0:H - 1], start=False, stop=False)
                    nc.tensor.matmul(z[:, 0:H - 1], wk_sb[2][:],
                                     xv[:, 1:H], start=False, stop=True)
                    nc.scalar.copy(zs[:, H * wb:H * (wb + 1)], z[:])

                u_t = up.tile([TOPK, COUT * W], f32)
                u_view = u_t[:].rearrange("p (o wb w8) -> p o wb w8",
                                          o=COUT, wb=NWB, w8=8)
                for wb in range(NWB):
                    zsl = zs[:, H * wb:H * (wb + 1)]
                    # top-56 via 7 rounds of max8 + match_replace masking
                    v = vp.tile([128, NV], f32)
                    nc.vector.max(v[:, 0:8], zsl[:])
                    for r in range(1, 7):
                        nc.vector.match_replace(zsl[:], v[:, 8 * r - 8:8 * r],
                                                zsl[:], -1e30)
                        nc.vector.max(v[:, 8 * r:8 * r + 8], zsl[:])

                    # tanh(top50 + bias)
                    res = resp.tile([128, TOPK], f32)
                    nc.scalar.activation(res[:], v[:, 0:TOPK],
                                         mybir.ActivationFunctionType.Tanh,
                                         bias=bias_sb[:, 0:1])

                    # transpose [(o,w8), r] -> [r, (o,w8)] for w-contiguous store
                    oT = otpsum.tile([TOPK, 128], f32)
                    nc.tensor.transpose(oT[:], res[:], ident_sb[:, :])
                    nc.scalar.copy(
                        u_view[:, :, wb, :],
                        oT[:].rearrange("p (o w8) -> p o w8", o=COUT, w8=8),
                    )

                nc.sync.dma_start(
                    out[s].rearrange("o r w -> r o w"),
                    u_t[:].rearrange("p (o w) -> p o w", o=COUT),
                )

    nc.compile()
    return nc


def _prep_consts(conv_w, conv_b):
    conv_w = np.asarray(conv_w, dtype=np.float32)
    conv_b = np.asarray(conv_b, dtype=np.float32)
    wmat = conv_w[:, 0, :, 0]  # [COUT, KH]
    wkj = np.zeros((KH, 8, 128), dtype=np.float32)
    for k in range(KH):
        for o in range(COUT):
            for w8 in range(8):
                wkj[k, w8, o * 8 + w8] = wmat[o, k]
    bias_p = np.repeat(conv_b, 8).astype(np.float32)[:, None]  # [(o,w8), 1]
    ident = np.eye(128, dtype=np.float32)
    return wkj.reshape(KH * 8, 128), bias_p, ident


def get_compiled(n_samples=BS):
    key = n_samples
    if key not in _CACHE:
        _CACHE[key] = build_module(n_samples)
    return _CACHE[key]


def kernel(x, conv_w, conv_b):
    from concourse.bass_utils import run_bass_kernel_spmd

    x = np.asarray(x, dtype=np.float32)
    nc = get_compiled(BS)
    wkj, bias_p, ident = _prep_consts(conv_w, conv_b)

    xs = x.reshape(B, H, W)  # squeeze CIN=1
    in_maps = []
    for c in range(N_CORES):
        in_maps.append({
            "x": np.ascontiguousarray(xs[c * BS:(c + 1) * BS]),
            "wkj": wkj,
            "bias_p": bias_p,
            "ident": ident,
        })
    res = run_bass_kernel_spmd(nc, in_maps, list(range(N_CORES)))
    out = np.concatenate([res.results[c]["out"] for c in range(N_CORES)], axis=0)
    return out


# revision 18
# speedup vs baseline: 3380.3134x; 3380.3134x over previous
"""Trainium2 Bass kernel for CNNLayer: conv(K=3 along H) + bias + tanh + topk(50) along H.

Full input contract:
  x:      [1024, 1, 200, 32] f32
  conv_w: [16, 1, 3, 1]      f32
  conv_b: [16]               f32
Output: [1024, 16, 50, 32] f32 (top-50 along H, sorted descending, after tanh)

Strategy: data-parallel over batch across 8 cores (128 samples/core).
Per sample:
  - load x[s] in h32-partition layout, DVE stream-transpose to xT[w, h]
  - replicate-shift xT into xT3b[(k,w8), (wb, c)] via 12 small SBUF DMAs
    (c = h+1; tap shifts baked into column offsets; boundary cols zeroed)
  - conv as 2 PE matmuls contracting K=24=(k,w8) with block-diagonal weights
    -> z[(o,w8), (wb, h)] in PSUM (no padding, no accumulation groups)
  - top-k on raw z (tanh monotonic => topk commutes): 7x DVE max8 + 6x
    match_replace masking per (o,w8)-row over the 200 h values
  - tanh(top50 + bias[o]) on ACT
  - PE-transpose result to [r, (o,w8)] so the output DMA is w-contiguous
"""

import sys

sys.path.insert(0, "/opt/trn_rl_repo")

import numpy as np

N_CORES = 8
B, H, W = 1024, 200, 32
COUT, KH, TOPK = 16, 3, 50
BS = B // N_CORES  # samples per core
NWB = W // 8       # 4 w-blocks of 8 -> (o, w8) = 128 partition rows
NV = 56            # values extracted per row (7 rounds of 8)
CW = 202           # columns per wb block in xT3b (c = h+1, c in [0, 201])

_CACHE = {}


def build_module(n_samples=BS, bufs=None, topk_repeat=1):
    import concourse.bass as bass  # noqa: F401
    import concourse.tile as tile
    from concourse import bacc, mybir

    _bufs = dict(xin=12, xt=12, xt3=12, zpsum=4, zs=10, v=10, res=10,
                 otpsum=2, xtpsum=2, u=12)
    _bufs.update(bufs or {})
    bufs = _bufs
    f32 = mybir.dt.float32
    nc = bacc.Bacc("TRN2", target_bir_lowering=False, debug=False,
                   num_devices=N_CORES)

    x = nc.dram_tensor("x", [n_samples, H, W], f32, kind="ExternalInput").ap()
    wkj = nc.dram_tensor("wkj", [KH * 8, 128], f32, kind="ExternalInput").ap()
    bias = nc.dram_tensor("bias_p", [128, 1], f32, kind="ExternalInput").ap()
    ident = nc.dram_tensor("ident", [128, 128], f32, kind="ExternalInput").ap()
    out = nc.dram_tensor("out", [n_samples, COUT, TOPK, W], f32,
                         kind="ExternalOutput").ap()

    with tile.TileContext(nc) as tc:
        with (
            tc.tile_pool(name="const", bufs=1) as constp,
            tc.tile_pool(name="xin", bufs=bufs["xin"]) as xinp,
            tc.tile_pool(name="xt", bufs=bufs["xt"]) as xtp,
            tc.tile_pool(name="xt3", bufs=bufs["xt3"]) as xt3p,
            tc.tile_pool(name="zpsum", bufs=bufs["zpsum"], space="PSUM") as zpsum,
            tc.tile_pool(name="zs", bufs=bufs["zs"]) as zsp,
            tc.tile_pool(name="v", bufs=bufs["v"]) as vp,
            tc.tile_pool(name="res", bufs=bufs["res"]) as resp,
            tc.tile_pool(name="otpsum", bufs=bufs["otpsum"], space="PSUM") as otpsum,
            tc.tile_pool(name="xtpsum", bufs=bufs["xtpsum"], space="PSUM") as xtpsum,
            tc.tile_pool(name="u", bufs=bufs["u"]) as up,
        ):
            wk_sb = []
            for k in range(KH):
                wt = constp.tile([8, 128], f32, tag=f"wk{k}")
                nc.sync.dma_start(wt[:], wkj[8 * k:8 * (k + 1), :])
                wk_sb.append(wt)
            ident_sb = constp.tile([128, 128], f32)
            nc.sync.dma_start(ident_sb[:], ident[:])
            bias_sb = constp.tile([128, 1], f32)
            nc.sync.dma_start(bias_sb[:], bias[:])

            for s in range(n_samples):
                # load x[s]=[200,32] as [h(100), (half, w)]; PE-transpose
                # both halves into one PSUM tile -> xT[w, xrow]
                xin = xinp.tile([100, 64], f32)
                nc.sync.dma_start(
                    xin[:].rearrange("h (hh w) -> h hh w", hh=2),
                    x[s].rearrange("(hh h) w -> h hh w", hh=2),
                )
                xtps = xtpsum.tile([32, H], f32)
                nc.tensor.transpose(xtps[:, 0:100], xin[:, 0:32],
                                    ident_sb[:100, :100])
                nc.tensor.transpose(xtps[:, 100:200], xin[:, 32:64],
                                    ident_sb[:100, :100])
                xT = xtp.tile([32, H], f32)
                nc.scalar.copy(xT[:], xtps[:])

                # xT8[w8, (wb, j)] = x[j, 8wb+w8]: w-blocks into the free dim
                x8 = xt3p.tile([8, NWB * H], f32)
                for wb in range(NWB):
                    nc.sync.dma_start(x8[:, H * wb:H * (wb + 1)],
                                      xT[8 * wb:8 * wb + 8, :])

                # conv per wb: 3 accumulating matmuls (taps), K=8;
                # per-tap column ranges make the H-boundary exact (no pad)
                zs = zsp.tile([128, NWB * H], f32)
                for wb in range(NWB):
                    z = zpsum.tile([128, H], f32)
                    xv = x8[:, H * wb:H * (wb + 1)]
                    nc.tensor.matmul(z[:, 0:H], wk_sb[1][:],
                                     xv[:, 0:H], start=True, stop=False)
                    nc.tensor.matmul(z[:, 1:H], wk_sb[0][:],
                                     xv[:, 0:H - 1], start=False, stop=False)
                    nc.tensor.matmul(z[:, 0:H - 1], wk_sb[2][:],
                                     xv[:, 1:H], start=False, stop=True)
                    nc.scalar.copy(zs[:, H * wb:H * (wb + 1)], z[:])

                u_t = up.tile([TOPK, COUT * W], f32)
                u_view = u_t[:].rearrange("p (o wb w8) -> p o wb w8",
                                          o=COUT, wb=NWB, w8=8)
                # top-56 via 7 rounds of max8 + match_replace masking;
                # rounds interleaved across w-blocks so DVE alternates
                # independent chains (hides per-op write-ack latency)
                zsl = [zs[:, H * wb:H * (wb + 1)] for wb in range(NWB)]
                vt = []
                for wb in range(NWB):
                    v = vp.tile([128, NV], f32, tag=f"v{wb}")
                    nc.vector.max(v[:, 0:8], zsl[wb][:])
                    vt.append(v)
                for rep in range(topk_repeat):
                    for r in range(1, 7):
                        for wb in range(NWB):
                            nc.vector.match_replace(
                                zsl[wb][:], vt[wb][:, 8 * r - 8:8 * r],
                                zsl[wb][:], -1e30)
                            nc.vector.max(vt[wb][:, 8 * r:8 * r + 8],
                                          zsl[wb][:])

                for wb in range(NWB):
                    # tanh(top50 + bias)
                    res = resp.tile([128, TOPK], f32)
                    nc.scalar.activation(res[:], vt[wb][:, 0:TOPK],
                                         mybir.ActivationFunctionType.Tanh,
                                         bias=bias_sb[:, 0:1])

                    # transpose [(o,w8), r] -> [r, (o,w8)] for w-contiguous store
                    oT = otpsum.tile([TOPK, 128], f32)
                    nc.tensor.transpose(oT[:], res[:], ident_sb[:, :])
                    nc.scalar.copy(
                        u_view[:, :, wb, :],
                        oT[:].rearrange("p (o w8) -> p o w8", o=COUT, w8=8),
                    )

                nc.sync.dma_start(
                    out[s].rearrange("o r w -> r o w"),
                    u_t[:].rearrange("p (o w) -> p o w", o=COUT),
                )

    nc.compile()
    return nc


def _prep_consts(conv_w, conv_b):
    conv_w = np.asarray(conv_w, dtype=np.float32)
    conv_b = np.asarray(conv_b, dtype=np.float32)
    wmat = conv_w[:, 0, :, 0]  # [COUT, KH]
    wkj = np.zeros((KH, 8, 128), dtype=np.float32)
    for k in range(KH):
        for o in range(COUT):
            for w8 in range(8):
                wkj[k, w8, o * 8 + w8] = wmat[o, k]
    bias_p = np.repeat(conv_b, 8).astype(np.float32)[:, None]  # [(o,w8), 1]
    ident = np.eye(128, dtype=np.float32)
    return wkj.reshape(KH * 8, 128), bias_p, ident


def get_compiled(n_samples=BS):
    key = n_samples
    if key not in _CACHE:
        _CACHE[key] = build_module(n_samples)
    return _CACHE[key]


def kernel(x, conv_w, conv_b):
    from concourse.bass_utils import run_bass_kernel_spmd

    x = np.asarray(x, dtype=np.float32)
    nc = get_compiled(BS)
    wkj, bias_p, ident = _prep_consts(conv_w, conv_b)

    xs = x.reshape(B, H, W)  # squeeze CIN=1
    in_maps = []
    for c in range(N_CORES):
        in_maps.append({
            "x": np.ascontiguousarray(xs[c * BS:(c + 1) * BS]),
            "wkj": wkj,
            "bias_p": bias_p,
            "ident": ident,
        })
    res = run_bass_kernel_spmd(nc, in_maps, list(range(N_CORES)))
    out = np.concatenate([res.results[c]["out"] for c in range(N_CORES)], axis=0)
    return out
